# revision 9
# baseline (speedup 1.0000x reference)
"""Trainium2 Bass kernel for nn_LinearReg_55508157333593.

loss = (c_omega * 0.001 / N) * sum of L2 norms of all 25-float groups of
weight [100000, 800] f32.  The flat buffer is 3.2M consecutive 25-float
groups; we shard it across 8 NeuronCores (10M floats each, [128, 78125]
per core) and stream chunks through SBUF.

v2 pipeline (raw Bass, no Tile), drain-optimized:
  SP   streams chunk i into input ring slot (HWDGE, per-slot sems)
  ACT  squares each chunk in place (f32)
  DVE  per-group (25) sums via 3D-AP tensor_reduce:
        - bulk chunks -> gs columns (sqrt'ed on device)
        - late (raw) chunks -> pr columns directly (host applies sqrt)
        - final 25-float chunks: fused square+sum (tensor_tensor_reduce)
          straight from the raw input slot
  ACT  sqrt+accum (accum_out) over gs segments -> pr columns, all emitted
       mid-stream so no sqrt work sits in the post-stream drain.
Output per core: pr [128, ncol] = [seg accum cols | raw group sums].
The host sqrts the raw columns and does the final f64 summation (it
already summed across cores in v1).

Schedule: 76 x 1000-float chunks + a receipt-aware descending tail, so
after the last DMA byte only ~receipt + one tiny fused reduce + the out
DMA remain on the critical path.
"""

import sys

import numpy as np

if "/opt/trn_rl_repo" not in sys.path:
    sys.path.insert(0, "/opt/trn_rl_repo")

N_CORES = 8
P = 128
GROUP = 25
C_OMEGA = 0.001
N_ROWS = 100000
ROW = 800
F_PER_PART = (N_ROWS * ROW) // (N_CORES * P)   # 78125

SCHEDULE = [1000] * 76 + [700, 675, 350, 200, 100, 50, 25, 25]
N_TTR = 0                  # trailing 25-float fused-reduce chunks
RAW_FROM = 72              # chunks >= this ship raw group sums (host sqrts)
# sqrt segments over bulk gs: (end_chunk_exclusive, emit_after_sq_of_chunk)
SEGS = [(15, 17), (29, 31), (44, 46), (58, 60), (72, 74)]
IN_BUFS = 20

_compiled = None
LAST_RESULTS = None


def build(schedule=None, segs=None, raw_from=None, n_ttr=N_TTR,
          in_bufs=IN_BUFS):
    from concourse import bacc, mybir
    from concourse.alu_op_type import AluOpType

    if schedule is None:
        schedule, segs, raw_from = SCHEDULE, SEGS, RAW_FROM
    n = len(schedule)
    f_per_part = sum(schedule)
    assert all(s % GROUP == 0 for s in schedule)
    for i in range(n - n_ttr, n):
        assert schedule[i] == GROUP
    offs = [sum(schedule[:i]) for i in range(n)]
    gpcs = [s // GROUP for s in schedule]
    n_segs = len(segs)
    assert segs[-1][0] == raw_from
    bulk_g = sum(gpcs[:raw_from])
    ncol = n_segs + sum(gpcs[raw_from:])
    goffs = {}
    acc = 0
    for i in range(raw_from):
        goffs[i] = acc
        acc += gpcs[i]
    roffs = {}
    racc = n_segs
    for i in range(raw_from, n):
        roffs[i] = racc
        racc += gpcs[i]
    seg_bounds = [b for b, _ in segs]
    assert seg_bounds == sorted(seg_bounds) and seg_bounds[-1] == raw_from
    seg_gr = []
    prev = 0
    for b, _ in segs:
        seg_gr.append((goffs[prev], goffs[b] if b < raw_from else bulk_g))
        prev = b
    emit_after = {e: si for si, (_, e) in enumerate(segs)}
    assert len(emit_after) == n_segs

    f32 = mybir.dt.float32
    Act = mybir.ActivationFunctionType
    max_sz = max(schedule)

    nc = bacc.Bacc("TRN2", target_bir_lowering=False, debug=False,
                   num_devices=N_CORES)
    x = nc.dram_tensor("x", [P, f_per_part], f32, kind="ExternalInput").ap()
    out = nc.dram_tensor("out", [P, ncol], f32, kind="ExternalOutput").ap()

    B = in_bufs
    ring = nc.alloc_sbuf_tensor("ring", [P, B * max_sz], f32).ap()
    gs = nc.alloc_sbuf_tensor("gs", [P, bulk_g], f32).ap()
    pr = nc.alloc_sbuf_tensor("pr", [P, ncol], f32).ap()
    sqs = (nc.alloc_sbuf_tensor("sqs", [P, n_ttr * GROUP], f32).ap()
           if n_ttr else None)
    dm = nc.alloc_sbuf_tensor("dm_scratch", [1, 1], f32).ap()
    ones = nc.const_aps.aps[(f32, 1.0)]

    tslot = [ring[:, b * max_sz:(b + 1) * max_sz] for b in range(B)]

    dma_sems = [nc.alloc_semaphore(f"dma_sem{b}") for b in range(B)]
    out_sem = nc.alloc_semaphore("out_sem")
    sq_sem = nc.alloc_semaphore("sq_sem")       # +1 per ACT square
    red_sem = nc.alloc_semaphore("red_sem")     # +1 per chunk (sums written)
    sqrt_sem = nc.alloc_semaphore("sqrt_sem")   # +1 per seg accum readout

    def emit_sp(sp):
        for i in range(n):
            if i >= B:
                # slot free once the reduce covering it completed
                sp.wait_ge(red_sem, i - B + 1)
            sp.dma_start(
                tslot[i % B][:, :schedule[i]],
                x[:, offs[i]:offs[i] + schedule[i]],
            ).then_inc(dma_sems[i % B], 16)
        sp.wait_ge(red_sem, n)
        sp.wait_ge(sqrt_sem, n_segs)
        sp.dma_start(out, pr).then_inc(out_sem, 16)
        sp.wait_ge(out_sem, 16)

    def emit_act(act):
        # table prefetch: Sqrt first => one table set covers Square too
        act.activation(dm, ones[0:1, :], Act.Sqrt)

        def emit_seg(si):
            b, _ = segs[si]
            glo, ghi = seg_gr[si]
            act.wait_ge(red_sem, b)
            act.activation(gs[:, glo:ghi], gs[:, glo:ghi], Act.Sqrt,
                           accum_out=pr[:, si:si + 1]).then_inc(sqrt_sem, 1)

        for i in range(n - n_ttr):
            act.wait_ge(dma_sems[i % B], 16 * (i // B + 1))
            s = schedule[i]
            act.activation(tslot[i % B][:, :s], tslot[i % B][:, :s],
                           Act.Square).then_inc(sq_sem, 1)
            if i in emit_after:
                emit_seg(emit_after[i])

    def emit_dve(dve):
        for i in range(n):
            s = schedule[i]
            g = gpcs[i]
            if i < raw_from:
                dst = gs[:, goffs[i]:goffs[i] + g]
            else:
                dst = pr[:, roffs[i]:roffs[i] + g]
            if i >= n - n_ttr:
                # fused square+sum from the raw input slot
                dve.wait_ge(dma_sems[i % B], 16 * (i // B + 1))
                j = i - (n - n_ttr)
                dve.tensor_tensor_reduce(
                    sqs[:, j * GROUP:(j + 1) * GROUP],
                    tslot[i % B][:, :GROUP], tslot[i % B][:, :GROUP],
                    1.0, 0.0, AluOpType.mult, AluOpType.add,
                    accum_out=dst,
                ).then_inc(red_sem, 1)
            else:
                dve.wait_ge(sq_sem, i + 1)
                dve.tensor_reduce(
                    dst,
                    tslot[i % B][:, :s].rearrange("p (g k) -> p g k", k=GROUP),
                    axis=mybir.AxisListType.X, op=AluOpType.add,
                ).then_inc(red_sem, 1)

    emit_sp(nc.sync)
    emit_act(nc.scalar)
    emit_dve(nc.vector)

    nc.compile()
    meta = dict(schedule=schedule, segs=segs, raw_from=raw_from,
                n_segs=n_segs, ncol=ncol, f_per_part=f_per_part)
    return nc, meta


def kernel(weight, c_omega):
    global _compiled, LAST_RESULTS
    from concourse.bass_utils import run_bass_kernel_spmd

    if _compiled is None:
        _compiled = build()
    nc, meta = _compiled

    w = np.asarray(weight)
    if w.dtype != np.float32:
        w = w.astype(np.float32)
    w = np.ascontiguousarray(w)
    flat = w.reshape(-1)
    per_core = flat.size // N_CORES
    in_maps = [
        {"x": flat[c * per_core:(c + 1) * per_core].reshape(P, F_PER_PART)}
        for c in range(N_CORES)
    ]
    LAST_RESULTS = run_bass_kernel_spmd(nc, in_maps,
                                        core_ids=list(range(N_CORES)))
    S = meta["n_segs"]
    total = 0.0
    for r in LAST_RESULTS.results:
        o = r["out"].astype(np.float64)
        total += o[:, :S].sum() + np.sqrt(o[:, S:]).sum()
    loss = total / N_ROWS * (C_OMEGA * float(c_omega))
    return np.float32(loss)


def selftest_sim():
    """CoreSim check on a scaled-down instance; returns rel err."""
    from concourse.bass_interp import CoreSim

    schedule = [250, 250, 150, 75, 50, 25, 25]
    segs = [(1, 2), (3, 4)]
    raw_from = 3
    f = sum(schedule)
    nc, meta = build(schedule=schedule, segs=segs, raw_from=raw_from,
                     in_bufs=3)
    rng = np.random.default_rng(0)
    xv = rng.standard_normal((P, f)).astype(np.float32)
    sim = CoreSim(nc)
    sim.tensor("x")[:] = xv
    sim.simulate()
    o = np.array(sim.tensor("out")).astype(np.float64)
    S = meta["n_segs"]
    got = o[:, :S].sum() + np.sqrt(o[:, S:]).sum()
    g = xv.reshape(P, f // GROUP, GROUP).astype(np.float64)
    want = np.sqrt((g ** 2).sum(-1)).sum()
    return abs(got - want) / abs(want)
